# revision 38
# baseline (speedup 1.0000x reference)
# GIN encoder (2x GINConv + BN + global_add_pool) on 8 Trainium2 NeuronCores.
#
# Sharding: nodes and edges are partitioned by destination-node owner
# (12500 nodes/core). Edges are grouped per 128-dst-node block and per
# 32768-row source range (dma_gather has int16 indices). Gathered source
# features (stored as a bf16 hi/lo pair, 256B rows) are scattered into
# per-block PSUM windows with one-hot matmuls; the GIN MLP consumes the
# hi|lo PSUM block with vertically stacked weights ([W1;W1]) so the pair
# recombination is free. BN stats go through an AllReduce; layer-1 outputs
# are re-split into bf16 pairs, transposed to node-major, and AllGathered
# so layer 2 can gather them. Pooling is a one-hot matmul over the sorted
# batch vector; per-core partial pools are overlap-added on the host.

import bisect
import hashlib
import numpy as np
import ml_dtypes

N_NODES = 100000
N_EDGES = 1000000
D = 64
NUM_GRAPHS = 512
BN_EPS = 1e-5

N_CORES = 8
P = 128
N_LOC = N_NODES // N_CORES          # 12500
NB = (N_LOC + P - 1) // P           # 98 blocks/core
N_PAD = NB * P                      # 12544
RANGE = 32768
N_RANGES = 4
GROUP_BLOCKS = 8                    # blocks per gather-call group
NG = (NB + GROUP_BLOCKS - 1) // GROUP_BLOCKS  # 7 stgroups
ST_BLOCKS = 4                       # blocks per PSUM supertile
CB = 8                              # one-hot chunks built per DVE op
CALL_CHUNKS = 8                     # max 128-slot chunks per dma_gather call

BF16 = ml_dtypes.bfloat16

_cache = {}


def _pair(x32):
    hi = x32.astype(BF16)
    lo = (x32 - hi.astype(np.float32)).astype(BF16)
    return np.concatenate([hi, lo], axis=1)


def _wrap16(vals):
    # dma_gather index layout: slot i -> [partition i%16, free i//16], x8 copies
    n = vals.shape[0]
    assert n % 16 == 0
    blk = vals.astype(np.int16).reshape(n // 16, 16).T  # [16, n//16]
    return np.tile(blk, (8, 1))  # [128, n//16]


def _prep_structure(edge_index, batch):
    src = np.asarray(edge_index[0], dtype=np.int64)
    dst = np.asarray(edge_index[1], dtype=np.int64)
    batch = np.asarray(batch, dtype=np.int64)

    owner = dst // N_LOC
    dst_loc = dst % N_LOC
    block = dst_loc // P
    loc = dst_loc % P
    g_of_b = block // GROUP_BLOCKS

    # gather-row ids per layer
    row_l = [src, (src // N_LOC) * N_PAD + (src % N_LOC)]
    n_rows_l = [N_NODES, N_CORES * N_PAD]

    layers = []
    for L in range(2):
        rows = row_l[L]
        rng = rows // RANGE
        # sort edges by (core, stgroup, range, block, row)
        order = np.lexsort((rows, block, rng, g_of_b, owner))
        so, sb, sr, srow, sloc = (
            owner[order], block[order], rng[order], rows[order], loc[order])

        # counts per (core, block, range)
        cnt = np.zeros((N_CORES, NB, N_RANGES), dtype=np.int64)
        np.add.at(cnt, (so, sb, sr), 1)
        chunks_br = (cnt.max(axis=0) + P - 1) // P  # [NB, 4] shared

        # group boundaries in the sorted edge array, keyed in sort order
        sg = sb // GROUP_BLOCKS
        key = ((so * NG + sg) * N_RANGES + sr) * NB + sb
        bounds = np.searchsorted(
            key, np.arange(N_CORES * NG * N_RANGES * NB + 1))

        total_chunks = int(chunks_br.sum())
        total_slots = total_chunks * P

        # slot offsets in (g, r, b, j) order; calls capped at CALL_CHUNKS
        # chunks (the SWDGE descriptor ring cannot hold more per op)
        slot_off = np.zeros((NB, N_RANGES), dtype=np.int64)
        chunk_off = np.zeros((NB, N_RANGES), dtype=np.int64)
        calls = []  # (g, r, slot_lo, slot_hi)
        pos = 0
        cpos = 0
        for g in range(NG):
            blo, bhi = g * GROUP_BLOCKS, min((g + 1) * GROUP_BLOCKS, NB)
            for r in range(N_RANGES):
                lo = pos
                for b in range(blo, bhi):
                    slot_off[b, r] = pos
                    chunk_off[b, r] = cpos
                    pos += int(chunks_br[b, r]) * P
                    cpos += int(chunks_br[b, r])
                while lo < pos:
                    hi = min(lo + CALL_CHUNKS * P, pos)
                    calls.append((g, r, lo, hi))
                    lo = hi
        assert pos == total_slots

        # matmul chunk order: block-major so each PSUM window's accumulation
        # group completes before the next one starts in the same bank
        call_bounds = [(lo, hi) for (_, _, lo, hi) in calls]
        mm_chunks = []
        for g in range(NG):
            blo, bhi = g * GROUP_BLOCKS, min((g + 1) * GROUP_BLOCKS, NB)
            for b in range(blo, bhi):
                for r in range(N_RANGES):
                    for j in range(int(chunks_br[b, r])):
                        s0 = int(slot_off[b, r]) + j * P
                        cid = bisect.bisect_right(
                            [l for (l, h) in call_bounds], s0) - 1
                        clo, chi = call_bounds[cid]
                        assert clo <= s0 < chi
                        mm_chunks.append((g, b, cid, (s0 - clo) // P, s0))

        # per-core slot arrays
        idx16_cores, gloc_cores = [], []
        for k in range(N_CORES):
            rows_sl = np.zeros(total_slots, dtype=np.int64)
            gloc_sl = np.full(total_slots, 255, dtype=np.int64)
            for b in range(NB):
                for r in range(N_RANGES):
                    gi = ((k * NG + b // GROUP_BLOCKS) * N_RANGES + r) * NB + b
                    e0, e1 = bounds[gi], bounds[gi + 1]
                    n = e1 - e0
                    s0 = slot_off[b, r]
                    cap = int(chunks_br[b, r]) * P
                    assert n <= cap
                    rows_sl[s0:s0 + n] = srow[e0:e1]
                    gloc_sl[s0:s0 + n] = sloc[e0:e1]
                    # pads: dummy valid row inside the same range
                    dummy = srow[e1 - 1] if n > 0 else r * RANGE
                    rows_sl[s0 + n:s0 + cap] = dummy
            # per-call int16 local indices
            parts = []
            for (g, r, lo, hi) in calls:
                v = rows_sl[lo:hi] - r * RANGE
                assert v.min() >= 0 and v.max() < RANGE
                parts.append(_wrap16(v))
            idx16_cores.append(np.concatenate(parts, axis=1))
            # gloc columns in matmul (block-major) chunk order
            ga = np.empty((total_chunks, P), dtype=np.int64)
            for ci, (_, _, _, _, s0) in enumerate(mm_chunks):
                ga[ci] = gloc_sl[s0:s0 + P]
            gloc_cores.append(ga.T.astype(BF16))  # [128, NCH]

        layers.append(dict(
            chunks_br=chunks_br, slot_off=slot_off, chunk_off=chunk_off,
            calls=calls, total_chunks=total_chunks, total_slots=total_slots,
            idx16=idx16_cores, gloc=gloc_cores, n_rows=n_rows_l[L],
            mm_chunks=mm_chunks,
        ))

    # pooling: per-core graph windows
    graph_base = []
    ploc_cores = []
    for k in range(N_CORES):
        bs = batch[k * N_LOC:(k + 1) * N_LOC]
        gb = int(bs[0]) if bs.size else 0
        pl = bs - gb
        assert pl.min() >= 0 and pl.max() < P, "graph window exceeds 128"
        plp = np.full(N_PAD, 255, dtype=np.int64)
        plp[:N_LOC] = pl
        graph_base.append(gb)
        ploc_cores.append(plp.reshape(NB, P).T.astype(BF16))  # [128, NB]

    return dict(layers=layers, graph_base=graph_base, ploc=ploc_cores)


def _build_program(struct, skip_cc=False, max_groups=None, skip_tail=False, max_layers=2, skip_mm=False, gather_only=False, skip_post=False, post_level=4):
    import concourse.bass as bass
    import concourse.tile as tile
    from concourse import bacc, mybir
    from concourse.masks import make_identity

    FP32 = mybir.dt.float32
    BF = mybir.dt.bfloat16
    I16 = mybir.dt.int16
    AOT = mybir.AluOpType
    ACT = mybir.ActivationFunctionType

    L0, L1 = struct["layers"]
    nc = bacc.Bacc("TRN2", target_bir_lowering=False, debug=False,
                   num_devices=N_CORES)

    # ---- I/O tensors ----
    x_pair_t = nc.dram_tensor("x_pair", [N_NODES, 2 * D], BF, kind="ExternalInput")
    xT_own_t = nc.dram_tensor("xT_own", [D, N_PAD], FP32, kind="ExternalInput")
    idx_t = [nc.dram_tensor(f"idx_l{i}", [P, Ld["idx16"][0].shape[1]], I16,
                            kind="ExternalInput") for i, Ld in enumerate((L0, L1))]
    gloc_t = [nc.dram_tensor(f"gloc_l{i}", [P, Ld["total_chunks"]], BF,
                             kind="ExternalInput") for i, Ld in enumerate((L0, L1))]
    ploc_t = nc.dram_tensor("ploc", [P, NB], BF, kind="ExternalInput")
    w1s_t = [nc.dram_tensor(f"w1s_{i}", [2 * D, D], FP32, kind="ExternalInput")
             for i in range(2)]
    w2_t = [nc.dram_tensor(f"w2_{i}", [D, D], FP32, kind="ExternalInput")
            for i in range(2)]
    b1_t = [nc.dram_tensor(f"b1_{i}", [D, 1], FP32, kind="ExternalInput")
            for i in range(2)]
    b2_t = [nc.dram_tensor(f"b2_{i}", [D, 1], FP32, kind="ExternalInput")
            for i in range(2)]
    gam_t = [nc.dram_tensor(f"gamma_{i}", [D, 1], FP32, kind="ExternalInput")
             for i in range(2)]
    bet_t = [nc.dram_tensor(f"beta_{i}", [D, 1], FP32, kind="ExternalInput")
             for i in range(2)]
    out_t = nc.dram_tensor("pool", [P, 2 * D], FP32, kind="ExternalOutput")

    # internal DRAM
    x0p_own = nc.dram_tensor("x0p_own", [N_PAD, 2 * D], BF)
    x0p_full = nc.dram_tensor("x0p_full", [N_CORES * N_PAD, 2 * D], BF,
                              addr_space="Local" if skip_cc else "Shared")
    bn_in = [nc.dram_tensor(f"bn_in_{i}", [D, 2], FP32) for i in range(2)]
    bn_out = [nc.dram_tensor(f"bn_out_{i}", [D, 2], FP32, addr_space="Shared")
              for i in range(2)]

    NST = (NB + ST_BLOCKS - 1) // ST_BLOCKS  # 25 supertiles

    with tile.TileContext(nc) as tc:
        with tc.tile_pool(name="const", bufs=1) as cpool, \
             tc.tile_pool(name="big", bufs=1) as bigp, \
             tc.tile_pool(name="gbuf", bufs=22) as gpool, \
             tc.tile_pool(name="work", bufs=3) as wpool, \
             tc.tile_pool(name="oh", bufs=3) as ohpool, \
             tc.tile_pool(name="psA", bufs=4, space="PSUM") as psA, \
             tc.tile_pool(name="psB", bufs=2, space="PSUM") as psB, \
             tc.tile_pool(name="psC", bufs=1, space="PSUM") as psC:

            # ---- constants ----
            iota_i = cpool.tile([P, CB * P], mybir.dt.int32)
            nc.gpsimd.iota(iota_i[:], pattern=[[0, CB], [1, P]], base=0,
                           channel_multiplier=0)
            iota_b = cpool.tile([P, CB * P], BF)
            nc.vector.tensor_copy(iota_b[:], iota_i[:])
            ident = cpool.tile([D, D], BF)
            make_identity(nc, ident[:])
            eps_t = cpool.tile([D, 1], FP32)
            nc.vector.memset(eps_t[:], BN_EPS)
            ploc_sb = cpool.tile([P, NB], BF)
            nc.sync.dma_start(ploc_sb[:], ploc_t.ap()[:, :])
            w1s_sb, w2_sb, b1_sb, b2_sb, gam_sb, bet_sb = [], [], [], [], [], []
            for i in range(2):
                t = cpool.tile([2 * D, D], FP32, tag="w1s")
                nc.sync.dma_start(t[:], w1s_t[i].ap()[:, :]); w1s_sb.append(t)
                t = cpool.tile([D, D], FP32, tag="w2")
                nc.sync.dma_start(t[:], w2_t[i].ap()[:, :]); w2_sb.append(t)
                for lst, tt, tag in ((b1_sb, b1_t, "b1"), (b2_sb, b2_t, "b2"),
                                     (gam_sb, gam_t, "gm"), (bet_sb, bet_t, "bt")):
                    t = cpool.tile([D, 1], FP32, tag=tag)
                    nc.sync.dma_start(t[:], tt[i].ap()[:, :]); lst.append(t)

            # persistent activations
            hT_all = bigp.tile([D, N_PAD], FP32, tag="hT")
            xnT_all = bigp.tile([D, N_PAD], FP32, tag="xnT")
            pool_acc = [bigp.tile([P, 2 * D], FP32, tag=f"pa{i}", name=f"pa{i}")
                        for i in range(2)]
            for i in range(2):
                nc.vector.memset(pool_acc[i][:], 0.0)

            gb_cols_max = max(
                max((hi - lo) // P for (_, _, lo, hi) in Ld["calls"])
                for Ld in (L0, L1))
            idx_cols_max = max(
                max((hi - lo) // 16 for (_, _, lo, hi) in Ld["calls"])
                for Ld in (L0, L1))

            def layer(Li, Ld):
                chunks_br = Ld["chunks_br"]
                calls = Ld["calls"]
                gloc_sb = wpool.tile([P, Ld["total_chunks"], 1], BF, tag="gloc", bufs=1)
                nc.sync.dma_start(gloc_sb[:, :, 0], gloc_t[Li].ap()[:, :])

                if Li == 0:
                    table = x_pair_t.ap()
                else:
                    table = x0p_full.ap()
                n_rows = Ld["n_rows"]

                # per-call metadata: idx dram column offsets
                call_info = []
                idx_col_off = 0
                for (g, r, lo, hi) in calls:
                    call_info.append((g, r, lo, hi, idx_col_off))
                    idx_col_off += (hi - lo) // 16

                s1p = wpool.tile([D, NST], FP32, tag="s1p")
                s2p = wpool.tile([D, NST], FP32, tag="s2p")
                gci = [0]  # running global chunk index (matmul order)
                call_tile = {}

                ngrun = NG if max_groups is None else min(NG, max_groups)
                for g in range(ngrun):
                    blo, bhi = g * GROUP_BLOCKS, min((g + 1) * GROUP_BLOCKS, NB)
                    # gathers for this group (one call per <=CALL_CHUNKS chunks)
                    for cid, (cg, r, lo, hi, ico) in enumerate(call_info):
                        if cg != g:
                            continue
                        S = hi - lo
                        it = wpool.tile([P, idx_cols_max], I16, tag="idx",
                                        bufs=6)
                        nc.sync.dma_start(
                            it[:, :S // 16],
                            idx_t[Li].ap()[:, ico:ico + S // 16])
                        gt = gpool.tile([P, CALL_CHUNKS, 2 * D], BF, tag="gb")
                        base = r * RANGE
                        nrows_r = min(RANGE, n_rows - base)
                        nc.gpsimd.dma_gather(
                            gt[:, :S // P, :],
                            table[base:base + nrows_r, :],
                            it[:, :S // 16],
                            S, S, 2 * D,
                        )
                        call_tile[cid] = gt
                        if gather_only:
                            nc.vector.tensor_tensor(
                                out=pool_acc[0][:, 0:D],
                                in0=pool_acc[0][:, 0:D],
                                in1=gt[:, 0, 0:D], op=AOT.add)
                    if gather_only:
                        continue

                    # chunks for this group, block-major (matmul order)
                    chl = [c for c in Ld["mm_chunks"] if c[0] == g]
                    ci0 = gci[0]
                    # psum tiles for this group's supertiles
                    sts = sorted(set(b // ST_BLOCKS for b in range(blo, bhi)))
                    stp = {st: psA.tile([P, ST_BLOCKS * P], FP32, tag="agg",
                                        name=f"agg{st}")
                           for st in sts}
                    nch_b = {b: int(chunks_br[b, :].sum()) for b in range(blo, bhi)}
                    seen_b = {b: 0 for b in range(blo, bhi)}

                    # one-hot tiles in CB batches, gloc columns follow matmul order
                    oh_tiles = []
                    ng_ch = len(chl)
                    for cb0 in range(0, ng_ch, CB):
                        n = min(CB, ng_ch - cb0)
                        oh = ohpool.tile([P, CB, P], BF, tag="oh")
                        nc.vector.tensor_tensor(
                            out=oh[:, :n, :],
                            in0=iota_b[:].rearrange("p (c s) -> p c s", c=CB)[:, :n, :],
                            in1=gloc_sb[:, ci0 + cb0:ci0 + cb0 + n, :]
                                .to_broadcast([P, n, P]),
                            op=AOT.is_equal,
                        )
                        oh_tiles.append(oh)

                    for ci, (_, b, cid, col, _) in enumerate(chl):
                        gt = call_tile[cid]
                        oh = oh_tiles[ci // CB]
                        st = b // ST_BLOCKS
                        win = (b % ST_BLOCKS) * P
                        first = seen_b[b] == 0
                        last = seen_b[b] == nch_b[b] - 1
                        seen_b[b] += 1
                        if skip_mm:
                            if first:
                                nc.tensor.matmul(
                                    stp[st][:, win:win + P],
                                    lhsT=gt[:, col, :],
                                    rhs=oh[:, ci % CB, :],
                                    start=True, stop=True,
                                )
                            continue
                        nc.tensor.matmul(
                            stp[st][:, win:win + P],
                            lhsT=gt[:, col, :],
                            rhs=oh[:, ci % CB, :],
                            start=first, stop=last,
                        )
                    gci[0] += ng_ch

                    # supertile post-processing: copy, MLP, h
                    if skip_post:
                        for st in sts:
                            nc.vector.tensor_tensor(
                                out=pool_acc[0][:], in0=pool_acc[0][:],
                                in1=stp[st][:, 0:2 * D], op=AOT.add)
                        continue
                    for st in sts:
                        sb0 = st * ST_BLOCKS
                        nwin = min(ST_BLOCKS, NB - sb0) * P
                        c0, c1 = sb0 * P, sb0 * P + nwin
                        agg_sb = wpool.tile([P, ST_BLOCKS * P], FP32, tag="aggsb", bufs=2)
                        nc.scalar.copy(agg_sb[:, :nwin], stp[st][:, :nwin])
                        if post_level < 2:
                            nc.vector.tensor_tensor(
                                out=pool_acc[0][:], in0=pool_acc[0][:],
                                in1=agg_sb[:, 0:2 * D], op=AOT.add)
                            continue
                        h1p = psB.tile([D, ST_BLOCKS * P], FP32, tag="mlp")
                        nc.tensor.matmul(h1p[:, :nwin], lhsT=w1s_sb[Li][:],
                                         rhs=agg_sb[:, :nwin],
                                         start=True, stop=False)
                        if Li == 0:
                            xsl = wpool.tile([D, ST_BLOCKS * P], FP32, tag="xsl", bufs=2)
                            nc.sync.dma_start(xsl[:, :nwin],
                                              xT_own_t.ap()[:, c0:c1])
                            xr = xsl[:, :nwin]
                        else:
                            xr = xnT_all[:, c0:c1]
                        nc.tensor.matmul(h1p[:, :nwin],
                                         lhsT=w1s_sb[Li][0:D, :], rhs=xr,
                                         start=False, stop=True)
                        t1 = wpool.tile([D, ST_BLOCKS * P], FP32, tag="t1", bufs=2)
                        nc.scalar.activation(t1[:, :nwin], h1p[:, :nwin],
                                             ACT.Tanh, bias=b1_sb[Li][:],
                                             scale=1.0)
                        if post_level < 3:
                            nc.vector.tensor_tensor(
                                out=pool_acc[0][0:D, 0:D], in0=pool_acc[0][0:D, 0:D],
                                in1=t1[0:D, 0:D], op=AOT.add)
                            continue
                        h2p = psB.tile([D, ST_BLOCKS * P], FP32, tag="mlp")
                        nc.tensor.matmul(h2p[:, :nwin], lhsT=w2_sb[Li][:],
                                         rhs=t1[:, :nwin], start=True, stop=True)
                        nc.scalar.activation(hT_all[:, c0:c1], h2p[:, :nwin],
                                             ACT.Tanh, bias=b2_sb[Li][:],
                                             scale=1.0)
                        if post_level < 4:
                            continue
                        # stats partials (exclude padded tail nodes)
                        r1 = min(c1, N_LOC)
                        if c0 < N_LOC:
                            hsl = hT_all[:, c0:r1]
                            nc.vector.tensor_reduce(
                                s1p[:, st:st + 1], hsl, axis=mybir.AxisListType.X,
                                op=AOT.add)
                            scr = wpool.tile([D, ST_BLOCKS * P], FP32, tag="scr", bufs=2)
                            nc.vector.tensor_tensor(
                                out=scr[:, :r1 - c0], in0=hsl, in1=hsl,
                                op=AOT.mult)
                            nc.vector.tensor_reduce(
                                s2p[:, st:st + 1], scr[:, :r1 - c0],
                                axis=mybir.AxisListType.X, op=AOT.add)

                if skip_tail:
                    return
                # ---- BN ----
                s1 = wpool.tile([D, 1], FP32, tag="s1")
                s2 = wpool.tile([D, 1], FP32, tag="s2")
                nc.vector.tensor_reduce(s1[:], s1p[:], axis=mybir.AxisListType.X,
                                        op=AOT.add)
                nc.vector.tensor_reduce(s2[:], s2p[:], axis=mybir.AxisListType.X,
                                        op=AOT.add)
                bpack = wpool.tile([D, 2], FP32, tag="bpack")
                nc.vector.tensor_copy(bpack[:, 0:1], s1[:])
                nc.vector.tensor_copy(bpack[:, 1:2], s2[:])
                nc.sync.dma_start(bn_in[Li].ap()[:, :], bpack[:])
                if not skip_cc:
                    nc.gpsimd.collective_compute(
                        "AllReduce", AOT.add,
                        replica_groups=[list(range(N_CORES))],
                        ins=[bn_in[Li].ap().opt()],
                        outs=[bn_out[Li].ap().opt()],
                    )
                bng = wpool.tile([D, 2], FP32, tag="bng")
                nc.sync.dma_start(
                    bng[:],
                    (bn_in[Li] if skip_cc else bn_out[Li]).ap()[:, :])
                mu = wpool.tile([D, 1], FP32, tag="mu")
                nc.scalar.mul(mu[:], bng[:, 0:1], 1.0 / N_NODES)
                ex2 = wpool.tile([D, 1], FP32, tag="ex2")
                nc.scalar.mul(ex2[:], bng[:, 1:2], 1.0 / N_NODES)
                var = wpool.tile([D, 1], FP32, tag="var")
                nc.vector.tensor_tensor(out=var[:], in0=mu[:], in1=mu[:],
                                        op=AOT.mult)
                nc.vector.tensor_tensor(out=var[:], in0=ex2[:], in1=var[:],
                                        op=AOT.subtract)
                rstd = wpool.tile([D, 1], FP32, tag="rstd")
                nc.scalar.activation(rstd[:], var[:], ACT.Sqrt,
                                     bias=eps_t[:], scale=1.0)
                nc.vector.reciprocal(rstd[:], rstd[:])
                inv = wpool.tile([D, 1], FP32, tag="inv")
                nc.vector.tensor_tensor(out=inv[:], in0=rstd[:], in1=gam_sb[Li][:],
                                        op=AOT.mult)
                nbias = wpool.tile([D, 1], FP32, tag="nbias")
                nc.vector.tensor_tensor(out=nbias[:], in0=mu[:], in1=inv[:],
                                        op=AOT.mult)
                nc.vector.tensor_tensor(out=nbias[:], in0=bet_sb[Li][:],
                                        in1=nbias[:], op=AOT.subtract)
                nc.vector.tensor_scalar(
                    out=xnT_all[:, :], in0=hT_all[:, :],
                    scalar1=inv[:], scalar2=nbias[:],
                    op0=AOT.mult, op1=AOT.add)

                # ---- pair split + transpose + pool (+ writeback for L0) ----
                for b in range(NB):
                    c0 = b * P
                    hi_b = wpool.tile([D, P], BF, tag="hib")
                    nc.scalar.copy(hi_b[:], xnT_all[:, c0:c0 + P])
                    lo_b = wpool.tile([D, P], BF, tag="lob")
                    nc.vector.tensor_tensor(out=lo_b[:],
                                            in0=xnT_all[:, c0:c0 + P],
                                            in1=hi_b[:], op=AOT.subtract)
                    tp = psC.tile([P, 2 * D], BF, tag="tp")
                    nc.tensor.transpose(tp[:, 0:D], hi_b[:], ident[:])
                    nc.tensor.transpose(tp[:, D:2 * D], lo_b[:], ident[:])
                    xp = wpool.tile([P, 2 * D], BF, tag="xp")
                    nc.scalar.copy(xp[:], tp[:])
                    if Li == 0:
                        nc.sync.dma_start(x0p_own.ap()[c0:c0 + P, :], xp[:])
                    # pool one-hot + matmul
                    poh = wpool.tile([P, P], BF, tag="poh")
                    nc.vector.tensor_tensor(
                        out=poh[:],
                        in0=iota_b[:, 0:P],
                        in1=ploc_sb[:, b:b + 1].to_broadcast([P, P]),
                        op=AOT.is_equal)
                    if b % ST_BLOCKS == 0:
                        pool_ps_cur = psC.tile([P, 2 * D], FP32, tag="pps")
                    nc.tensor.matmul(
                        pool_ps_cur[:], lhsT=poh[:], rhs=xp[:],
                        start=(b % ST_BLOCKS == 0),
                        stop=(b % ST_BLOCKS == ST_BLOCKS - 1 or b == NB - 1))
                    if b % ST_BLOCKS == ST_BLOCKS - 1 or b == NB - 1:
                        nc.vector.tensor_tensor(
                            out=pool_acc[Li][:], in0=pool_acc[Li][:],
                            in1=pool_ps_cur[:], op=AOT.add)

                if Li == 0 and not skip_cc:
                    nc.gpsimd.collective_compute(
                        "AllGather", AOT.bypass,
                        replica_groups=[list(range(N_CORES))],
                        ins=[x0p_own.ap().opt()],
                        outs=[x0p_full.ap().opt()],
                    )

            layer(0, L0)
            if max_layers > 1:
                layer(1, L1)

            # ---- final pool output ----
            osb = wpool.tile([P, 2 * D], FP32, tag="osb")
            for i in range(2):
                nc.vector.tensor_tensor(
                    out=osb[:, i * D:(i + 1) * D],
                    in0=pool_acc[i][:, 0:D], in1=pool_acc[i][:, D:2 * D],
                    op=AOT.add)
            nc.sync.dma_start(out_t.ap()[:, :], osb[:])

    nc.compile()
    return nc


def kernel(**inputs):
    from concourse.bass_utils import run_bass_kernel_spmd

    edge_index = np.asarray(inputs["edge_index"])
    batch = np.asarray(inputs["batch"])
    key = hashlib.sha1(
        edge_index.tobytes() + batch.tobytes()).hexdigest()
    if key not in _cache:
        struct = _prep_structure(edge_index, batch)
        nc = _build_program(struct)
        _cache[key] = (struct, nc)
    struct, nc = _cache[key]

    x = np.asarray(inputs["x"], dtype=np.float32)
    x_pair = _pair(x)
    in_maps = []
    for k in range(N_CORES):
        xT_own = np.zeros((D, N_PAD), dtype=np.float32)
        xT_own[:, :N_LOC] = x[k * N_LOC:(k + 1) * N_LOC].T
        m = dict(
            x_pair=x_pair,
            xT_own=xT_own,
            ploc=np.ascontiguousarray(struct["ploc"][k]),
        )
        for i, Ld in enumerate(struct["layers"]):
            m[f"idx_l{i}"] = np.ascontiguousarray(Ld["idx16"][k])
            m[f"gloc_l{i}"] = np.ascontiguousarray(Ld["gloc"][k])
        for i in range(2):
            W1 = np.asarray(inputs[f"W1_{i}"], dtype=np.float32)
            m[f"w1s_{i}"] = np.concatenate([W1, W1], axis=0)
            m[f"w2_{i}"] = np.asarray(inputs[f"W2_{i}"], dtype=np.float32)
            m[f"b1_{i}"] = np.asarray(inputs[f"b1_{i}"], dtype=np.float32).reshape(D, 1)
            m[f"b2_{i}"] = np.asarray(inputs[f"b2_{i}"], dtype=np.float32).reshape(D, 1)
            m[f"gamma_{i}"] = np.asarray(inputs[f"gamma_{i}"], dtype=np.float32).reshape(D, 1)
            m[f"beta_{i}"] = np.asarray(inputs[f"beta_{i}"], dtype=np.float32).reshape(D, 1)
        in_maps.append(m)

    res = run_bass_kernel_spmd(nc, in_maps, core_ids=list(range(N_CORES)))
    kernel.last_results = res

    out = np.zeros((NUM_GRAPHS, 2 * D), dtype=np.float32)
    for k in range(N_CORES):
        gb = struct["graph_base"][k]
        n = min(P, NUM_GRAPHS - gb)
        out[gb:gb + n] += res.results[k]["pool"][:n]
    return out
